# revision 17
# baseline (speedup 1.0000x reference)
"""Bass/Trainium2 kernel for nn_DirNet: per-direction EqualLinear over x[:, o, j, :].

Computes, for the full input x [256, 18, 18, 512], W [18, 512, 512], b [18, 512],
idx [18]:
    x_sel = x[:, :, idx, :]
    y = einsum('koji,odi->kojd', x_sel, W) * (1/sqrt(512)) + b
    out = x.at[:, :, idx, :].set(y)

Sharding: 8 cores as a (4 batch) x (2 direction-half) grid.
Each core handles x_loc [64, 9, 18, 512] with W_loc [9, 512, 512].

Host prep: W is pre-transposed to [o, i, d] and pre-scaled by SCALE (small,
19MB); x/y stay in natural layout and all heavy lifting runs on device.

Per-core device dataflow (Tile framework):
  - Row tiles of 128 = 64 batch x 2 layer positions (j-pairs), so M=128 exactly.
  - One 2.25MB DMA per direction loads all 18 j rows [128 part = (b, j%2), 9, 512].
  - The contraction dim i is moved to partitions with PE transposes
    (4 x 128x128 per row tile, float32r) packed into one PSUM bank, then one
    ACT copy to SBUF.
  - 4 accumulating float32r matmuls per row tile: psum[row, d] += xT_k.T @ WT_k.
  - bias (broadcast across partitions via DMA re-read) added during the
    PSUM->SBUF copy on DVE; one 2.25MB store per direction on the ACT ring.
"""
import contextlib
import math
import os
import sys

sys.path.insert(0, "/opt/trn_rl_repo")

import numpy as np

import concourse.bass as bass
import concourse.mybir as mybir
import concourse.tile as tile
from concourse import bacc
from concourse.masks import make_identity
from concourse.bass_utils import run_bass_kernel_spmd

# problem shape (hardcoded per contract)
B, O_DIM, J_DIM, D = 256, 18, 18, 512
N_CORES = 8
B_SHARD, O_SHARD = 4, 2
B_LOC, O_LOC = B // B_SHARD, O_DIM // O_SHARD  # 64, 9
N_RT = J_DIM // 2           # 9 row tiles per direction (128 rows = 64 b x 2 j)
KB = D // 128               # 4 contraction blocks
SCALE = 1.0 / math.sqrt(D)

F32 = mybir.dt.float32
DT_MM = mybir.dt.float32r   # PE matmul dtype: float32r = 1 cyc/row (vs fp32 4)

_nc_cache = {}


def build(dt_mm=DT_MM, loop_n=1, grp=None):
    GRP = int(os.environ.get("KGRP", "1")) if grp is None else grp
    key = (dt_mm, loop_n, GRP)
    if key in _nc_cache:
        return _nc_cache[key]
    nc = bacc.Bacc()
    X = nc.declare_dram_parameter("x", [B_LOC, O_LOC, J_DIM, D], dt_mm, isOutput=False)
    Wp = nc.declare_dram_parameter("wt", [O_LOC, D, D], dt_mm, isOutput=False)  # [o, i, d], pre-scaled
    Bp = nc.declare_dram_parameter("b", [O_LOC, D], F32, isOutput=False)
    Yp = nc.declare_dram_parameter("y", [B_LOC, O_LOC, J_DIM, D], F32, isOutput=True)

    # [64, 18, 512] rows for one direction map to [(b j2)=128, jc=9, 512] SBUF
    # tiles with partition p = b*2 + j%2.  A single DMA would need a 4D AP
    # (unbalanceable), so transfers are split into even/odd-j halves, each 3D.
    def dma_rows(dma_fn, sbuf_tile, T, o, to_sbuf):
        for j2 in range(2):
            dram = T[:, o, j2::2, :]          # [64, 9, 512]
            sb = sbuf_tile[j2::2]             # [64, 9, 512], partition step 2
            if to_sbuf:
                dma_fn(sb, dram)
            else:
                dma_fn(dram, sb)

    with tile.TileContext(nc) as tc:
        with tc.tile_pool(name="const", bufs=1) as const, \
             tc.tile_pool(name="wt", bufs=2) as wt_pool, \
             tc.tile_pool(name="xin", bufs=2) as xin_pool, \
             tc.tile_pool(name="xt", bufs=6) as xt_pool, \
             tc.tile_pool(name="yout", bufs=2) as y_pool, \
             tc.tile_pool(name="ps_xt", bufs=4, space="PSUM") as ps_xt, \
             tc.tile_pool(name="ps_y", bufs=4, space="PSUM") as ps_y:
            ident_f = const.tile([128, 128], F32)
            make_identity(nc, ident_f[:])
            ident = const.tile([128, 128], dt_mm)
            nc.vector.tensor_copy(ident[:], ident_f[:])

            # all 9 bias rows broadcast to 128 partitions, loaded once
            b_all = const.tile([128, O_LOC, D], F32)
            nc.gpsimd.dma_start(b_all[:], Bp[None, :, :].broadcast_to((128, O_LOC, D)))

            loop_cm = (tc.For_i(0, loop_n, 1,
                               hint_engines=(mybir.EngineType.PE,
                                             mybir.EngineType.DVE,
                                             mybir.EngineType.Activation))
                       if loop_n > 1 else contextlib.nullcontext())
            with loop_cm:
                # Software-pipelined over a flat list of (o, group) work items.
                # GRP row tiles share one PSUM transpose buffer and one ACT
                # PSUM->SBUF copy; transposes for item i+1 are emitted before
                # the matmuls of item i so the copy latency hides behind PE
                # transpose work instead of stalling the matmul stream.
                x_tiles, w_tiles, y_tiles = {}, {}, {}

                def start_o(o):
                    if o in x_tiles or o >= O_LOC:
                        return
                    # W_o^T [i, d] pre-transposed on host: one 1MB DMA (SWDGE)
                    wT = wt_pool.tile([128, KB, D], dt_mm, tag="wT")
                    nc.gpsimd.dma_start(
                        wT[:], Wp[o].rearrange("(kb p) d -> p kb d", p=128))
                    # all rows of this direction: 2 x 1.1MB DMAs (SP ring)
                    x_nat = xin_pool.tile([128, N_RT, D], dt_mm, tag="x")
                    dma_rows(nc.sync.dma_start, x_nat, X, o, to_sbuf=True)
                    w_tiles[o], x_tiles[o] = wT, x_nat

                def emit_transposes(o, g):
                    x_nat = x_tiles[o]
                    p_xt = ps_xt.tile([128, GRP, KB, 128], dt_mm, tag="p_xt")
                    for tt in range(GRP):
                        t = g * GRP + tt
                        for k in range(KB):
                            nc.tensor.transpose(p_xt[:, tt, k, :],
                                                x_nat[:, t, k * 128:(k + 1) * 128],
                                                ident[:])
                    xt = xt_pool.tile([128, GRP, KB, 128], dt_mm, tag="xt")
                    nc.scalar.activation(xt[:], p_xt[:],
                                         mybir.ActivationFunctionType.Copy)
                    return xt

                items = [(o, g) for o in range(O_LOC) for g in range(N_RT // GRP)]
                start_o(0)
                xts = {0: emit_transposes(*items[0])}
                for i, (o, g) in enumerate(items):
                    if g == 0:
                        start_o(o + 1)     # prefetch next direction's DMAs
                        y_tiles[o] = y_pool.tile([128, N_RT, D], F32, tag="y", name="y_o")
                    if i + 1 < len(items):
                        xts[i + 1] = emit_transposes(*items[i + 1])
                    xt = xts.pop(i)
                    y_o = y_tiles[o]
                    wT = w_tiles[o]
                    for tt in range(GRP):
                        t = g * GRP + tt
                        p_y = ps_y.tile([128, D], F32, tag="p_y")
                        for k in range(KB):
                            nc.tensor.matmul(p_y[:], xt[:, tt, k, :], wT[:, k, :],
                                             start=(k == 0), stop=(k == KB - 1))
                        # bias add + PSUM->SBUF on DVE
                        nc.vector.tensor_add(y_o[:, t, :], p_y[:], b_all[:, o, :])
                    if g == N_RT // GRP - 1:
                        # 2 x 1.1MB stores on the ACT ring
                        dma_rows(nc.scalar.dma_start, y_o, Yp, o, to_sbuf=False)
                        del x_tiles[o], w_tiles[o], y_tiles[o]
    nc.finalize()
    _nc_cache[key] = nc
    return nc


def prep_w(W):
    return np.ascontiguousarray(np.transpose(W * np.float32(SCALE), (0, 2, 1)))


def make_in_maps(x_sel, W, b, w_is_prepped=False):
    wt = W if w_is_prepped else prep_w(W)
    in_maps = []
    for c in range(N_CORES):
        bq, oh = divmod(c, O_SHARD)
        in_maps.append({
            "x": np.ascontiguousarray(
                x_sel[bq * B_LOC:(bq + 1) * B_LOC, oh * O_LOC:(oh + 1) * O_LOC]),
            "wt": np.ascontiguousarray(wt[oh * O_LOC:(oh + 1) * O_LOC]),
            "b": np.ascontiguousarray(b[oh * O_LOC:(oh + 1) * O_LOC]),
        })
    return in_maps


def gather_out(results):
    y = np.empty((B, O_DIM, J_DIM, D), dtype=np.float32)
    for c in range(N_CORES):
        bq, oh = divmod(c, O_SHARD)
        y[bq * B_LOC:(bq + 1) * B_LOC, oh * O_LOC:(oh + 1) * O_LOC] = results[c]["y"]
    return y


def kernel(x, W, b, idx):
    x = np.asarray(x, dtype=np.float32)
    W = np.asarray(W, dtype=np.float32)
    b = np.asarray(b, dtype=np.float32)
    idx = np.asarray(idx)

    identity_idx = bool(np.array_equal(idx, np.arange(J_DIM)))
    x_sel = x if identity_idx else np.ascontiguousarray(x[:, :, idx, :])

    nc = build()
    results = run_bass_kernel_spmd(nc, make_in_maps(x_sel, W, b),
                                   list(range(N_CORES))).results
    y = gather_out(results)

    if identity_idx:
        return y
    out = x.copy()
    out[:, :, idx, :] = y
    return out
